# revision 46
# baseline (speedup 1.0000x reference)
"""SpGAT_Conv Trainium2 kernel: 8-core SPMD spectral GNN conv.

Math (reference):
    a = softmax(alpha)
    pre = x @ W                                   [N, D]
    out_low  = s0 @ (a0 * (s1 @ pre))             [N, D]
    out_high = s2 @ (a1 * (s3 @ pre))             [N, D]
    out = relu(max(out_low, out_high) + bias)

Key restructure (v15): associativity.  t = S @ (x @ W) == (S @ x) @ W.
The first form needs pre = x@W for ALL N rows on every core (replicated:
256 extra matmuls/core; sharding it needs a pre-AllGather that can never
be hidden).  The second form needs only U_c = S_c @ x -- and x is an
INPUT, already replicated for free -- then t_c = U_c @ W over the core's
own 1024 t rows (32 matmuls).  Same total FLOPs, zero new communication:
per-core matmul count drops 1286 -> ~1062.

Sharding: row-shard the node dim N across 8 cores.  S = concat(s1, s3).
Core c owns rows [1024c, 1024c+1024):
    phase 2a: U^T accumulation, contraction over x rows, in two 512-row
              halves (U^T half = 4 dchunks x [128, 512] = 4 PSUM banks);
              lhsT = x chunks (natural row-major!), rhs = S_c^T strips.
    phase 2b: t_half = (U^T)^T @ W via 16 matmuls from SBUF-drained U^T;
              strips staged and AllGathered (groups [0-3], [4,5], [6,7])
              as soon as their rows stage.
    phase 3:  out_c = relu(max(a0*s0_c@t1 + b, a1*s2_c@t3 + b)), high band
              then low band, bias/a preloaded into PSUM (continue-mode
              accumulation), high stash relu'd eagerly so the epilogue is
              a single fused (a0*acc) max stash DVE op per strip.

Queue discipline (hard-won): DMA HW queue ring slots form one global
sequence shared by both HWDGE engines, so any DMA blocked on a collective
semaphore stalls every later-slotted DMA.  Hence (a) t chunks are loaded
JIT in exact phase-3 consumption order -- whatever such a load stalls
needed that same collective anyway -- and (b) those loads share a tile
pool with the phase-2 strip loads so buffer-reuse deps keep them from
occupying ring slots until this core's phase 2 is nearly done.

Compute dtype is bf16 (host-cast; full PE rate) with fp32 PSUM
accumulation.  A dependency-free tiny AllGather at kernel start absorbs
first-collective init + inter-core launch skew.
"""

import os

import numpy as np

N_CORES = 8
N = 8192
K = 2048
NK = N - K          # 6144
D = 512
ROWS = N // N_CORES  # 1024 rows per core
P = 128
RCH = ROWS // P      # 8  (row chunks per core / output strips)
NCH = N // P         # 64 (contraction chunks over full N)
KCH = K // P         # 16 (low-band chunks; high band = 48)
DCH = D // P         # 4  (dchunks)
HALF = 512           # U rows per phase-2 half (4 PSUM banks of U^T)

# sub-AllGather strip groups (each collective costs ~25-35us mostly
# size-independent; the last gather is a hard barrier so its group is kept
# small and triggered as early as possible)
GROUPS = [[0, 1, 2, 3], [4, 5], [6, 7]]
GBASE = [0, 32, 48]  # q-index base of each group (8 ranks * strips)

COMPUTE = os.environ.get("SPGAT_COMPUTE", "bf16")
DEBUG = os.environ.get("SPGAT_DEBUG", "0") == "1"

_CACHE = {}

# t-chunk catalog: sub-AG g delivers, for every rank c, its strips GROUPS[g]
# = global chunks j = 8c + s.  Phase 3 consumes chunks in arrival (q) order:
# q = GBASE[g] + c * len(GROUPS[g]) + s_idx.
ARRIVAL = [
    (8 * c + s, GBASE[g] + c * len(GROUPS[g]) + si, g, c)
    for g in range(len(GROUPS))
    for c in range(N_CORES)
    for si, s in enumerate(GROUPS[g])
]


def _build_nc(compute):
    import concourse.mybir as mybir
    import concourse.tile as tile
    from concourse import bacc

    f32 = mybir.dt.float32
    bf16 = mybir.dt.bfloat16
    f32r = mybir.dt.float32r
    cdt = bf16 if compute == "bf16" else f32

    def mmcast(ap):
        return ap.bitcast(f32r) if compute == "f32r" else ap

    nc = bacc.Bacc(
        "TRN2", target_bir_lowering=False, debug=False, num_devices=N_CORES
    )

    xn = nc.dram_tensor("xn", [N, D], cdt, kind="ExternalInput").ap()
    w = nc.dram_tensor("w", [D, D], cdt, kind="ExternalInput").ap()
    alpha = nc.dram_tensor("alpha", [2], f32, kind="ExternalInput").ap()
    bias = nc.dram_tensor("bias", [D], f32, kind="ExternalInput").ap()
    st = nc.dram_tensor("st", [N, ROWS], cdt, kind="ExternalInput").ap()
    s0t = nc.dram_tensor("s0t", [K, ROWS], cdt, kind="ExternalInput").ap()
    s2t = nc.dram_tensor("s2t", [NK, ROWS], cdt, kind="ExternalInput").ap()
    out = nc.dram_tensor("out", [ROWS, D], f32, kind="ExternalOutput").ap()
    if DEBUG:
        t_dump = nc.dram_tensor("t_dump", [N, D], cdt, kind="ExternalOutput").ap()

    groups = [list(range(N_CORES))]

    with tile.TileContext(nc) as tc:
        with (
            tc.tile_pool(name="const", bufs=1) as const,
            tc.tile_pool(name="bigA", bufs=1) as bigA,
            tc.tile_pool(name="strips", bufs=12) as strips,
            tc.tile_pool(name="rstrips", bufs=12) as rstrips,
            tc.tile_pool(name="ut", bufs=8) as utp,
            tc.tile_pool(name="stage", bufs=5) as stage,
            tc.tile_pool(name="stash", bufs=1) as stashp,
            tc.tile_pool(name="ps", bufs=8, space="PSUM") as ps,
            tc.tile_pool(name="dram", bufs=1, space="DRAM") as dram,
        ):
            # ---- collective warm-up: absorb first-collective init + skew
            warm_in = dram.tile([8, 8], f32, name="warm_in")
            warm_out = dram.tile([64, 8], f32, name="warm_out", addr_space="Shared")
            nc.gpsimd.collective_compute(
                "AllGather",
                mybir.AluOpType.bypass,
                replica_groups=groups,
                ins=[warm_in.opt()],
                outs=[warm_out.opt()],
            )

            # ---- input DMAs: tiny alpha/bias first (feed the setup
            # matmuls, first on the PE queue), then w, then x chunked
            asb = const.tile([1, 2], f32, name="asb")
            nc.sync.dma_start(asb[:], alpha[None, :])
            bsb = const.tile([1, D], f32, name="bsb")
            nc.sync.dma_start(bsb[:], bias[None, :])
            xn_v = xn.rearrange("(c p) d -> p c d", p=P)
            xn_sb = bigA.tile([P, NCH, D], cdt, name="xn_sb", tag="bigA")
            # only the first x pieces load up front; the rest interleave
            # with the phase-2a strip loads (issuing all 8MB here would queue
            # ahead of the first strip load and stall the PE ~30us).  w is
            # needed only by phase-2b (~80us in), so it loads after the
            # phase-2a critical-path pieces.
            nc.sync.dma_start(xn_sb[:, 0:4, :], xn_v[:, 0:4, :])
            nc.sync.dma_start(xn_sb[:, 4:8, :], xn_v[:, 4:8, :])
            w_sb = const.tile([P, DCH, D], cdt, name="w_sb")
            nc.sync.dma_start(w_sb[:], w.rearrange("(c p) d -> p c d", p=P))

            # ---- setup: softmax(alpha); broadcast a, bias/a0, bias/a1 to
            # 128 partitions via ones-matmul.
            amax = const.tile([1, 1], f32, name="amax")
            nc.vector.tensor_tensor(
                amax[:], asb[:, 0:1], asb[:, 1:2], mybir.AluOpType.max
            )
            ash = const.tile([1, 2], f32, name="ash")
            nc.vector.tensor_scalar(
                ash[:], asb[:], amax[:, 0:1], None, mybir.AluOpType.subtract
            )
            aexp = const.tile([1, 2], f32, name="aexp")
            nc.scalar.activation(aexp[:], ash[:], mybir.ActivationFunctionType.Exp)
            asum = const.tile([1, 1], f32, name="asum")
            nc.vector.tensor_tensor(
                asum[:], aexp[:, 0:1], aexp[:, 1:2], mybir.AluOpType.add
            )
            arec = const.tile([1, 1], f32, name="arec")
            nc.vector.reciprocal(arec[:], asum[:])
            afin = const.tile([1, 2], f32, name="afin")
            nc.vector.tensor_scalar(
                afin[:], aexp[:], arec[:, 0:1], None, mybir.AluOpType.mult
            )
            ainv = const.tile([1, 2], f32, name="ainv")
            nc.vector.reciprocal(ainv[:], afin[:])
            bd0 = const.tile([1, D], f32, name="bd0")
            nc.vector.tensor_scalar(
                bd0[:], bsb[:], ainv[:, 0:1], None, mybir.AluOpType.mult
            )
            bd1 = const.tile([1, D], f32, name="bd1")
            nc.vector.tensor_scalar(
                bd1[:], bsb[:], ainv[:, 1:2], None, mybir.AluOpType.mult
            )

            ones = const.tile([1, P], f32, name="ones")
            nc.vector.memset(ones[:], 1.0)
            zeros = const.tile([P, D], f32, name="zeros")
            nc.vector.memset(zeros[:], 0.0)
            ps_a = ps.tile([P, 2], f32, name="ps_a", tag="acc")
            nc.tensor.matmul(ps_a[:], ones[:], afin[:], start=True, stop=True)
            a128 = const.tile([P, 2], f32, name="a128")
            nc.vector.tensor_copy(a128[:], ps_a[:])
            ps_b = ps.tile([P, D], f32, name="ps_b", tag="acc")
            nc.tensor.matmul(ps_b[:], ones[:], bd0[:], start=True, stop=True)
            bd0_128 = const.tile([P, D], f32, name="bd0_128")
            nc.vector.tensor_copy(bd0_128[:], ps_b[:])
            ps_c = ps.tile([P, D], f32, name="ps_c", tag="acc")
            nc.tensor.matmul(ps_c[:], ones[:], bd1[:], start=True, stop=True)
            bd1_128 = const.tile([P, D], f32, name="bd1_128")
            nc.vector.tensor_copy(bd1_128[:], ps_c[:])

            # ---- phase 2: t_c = (S_c @ x) @ W in two 512-row halves
            t_in = dram.tile([ROWS, D], cdt, name="t_in")
            t_outs = [
                dram.tile([P * len(gs) * N_CORES, D], cdt, name=f"t_out{g}",
                          addr_space="Shared")
                for g, gs in enumerate(GROUPS)
            ]

            def t_subag(g):
                gs = GROUPS[g]
                nc.gpsimd.collective_compute(
                    "AllGather",
                    mybir.AluOpType.bypass,
                    replica_groups=groups,
                    ins=[t_in[P * gs[0] : P * (gs[-1] + 1), :].opt()],
                    outs=[t_outs[g].opt()],
                )

            for h in range(2):
                # --- 2a: U^T[dchunk][:, 512 rows] accumulation over x rows
                accU = [
                    ps.tile([P, HALF], f32, name=f"accU_{h}_{d}", tag="acc")
                    for d in range(DCH)
                ]
                for xc in range(NCH):
                    sl = strips.tile([P, HALF], cdt, name=f"st_{h}_{xc}",
                                     tag="strip")
                    nc.sync.dma_start(
                        sl[:],
                        st[P * xc : P * (xc + 1), HALF * h : HALF * (h + 1)],
                    )
                    if h == 0 and xc % 4 == 0 and xc // 4 + 2 < 16:
                        i = xc // 4 + 2  # stream the rest of x just ahead
                        nc.sync.dma_start(
                            xn_sb[:, 4 * i : 4 * (i + 1), :],
                            xn_v[:, 4 * i : 4 * (i + 1), :],
                        )
                    for d in range(DCH):
                        nc.tensor.matmul(
                            accU[d][:],
                            mmcast(xn_sb[:, xc, P * d : P * (d + 1)]),
                            mmcast(sl[:]),
                            start=(xc == 0),
                            stop=(xc == NCH - 1),
                        )
                uT = [
                    utp.tile([P, HALF], cdt, name=f"uT_{h}_{d}", tag="ut")
                    for d in range(DCH)
                ]
                for d in range(DCH):  # drain U^T to SBUF (bf16), alternate
                    if d % 2 == 0:
                        nc.vector.tensor_copy(uT[d][:], accU[d][:])
                    else:
                        nc.scalar.copy(uT[d][:], accU[d][:])
                # --- 2b: t rows [512h + 128r, +128) = U_half @ W
                for r in range(DCH):
                    acc = ps.tile([P, D], f32, name=f"acct_{h}_{r}", tag="acc")
                    for d in range(DCH):
                        nc.tensor.matmul(
                            acc[:],
                            mmcast(uT[d][:, P * r : P * (r + 1)]),
                            mmcast(w_sb[:, d, :]),
                            start=(d == 0),
                            stop=(d == DCH - 1),
                        )
                    tst = stage.tile([P, D], cdt, name=f"t_st_{h}_{r}", tag="st")
                    if r % 2 == 0:
                        nc.vector.tensor_copy(tst[:], acc[:])
                    else:
                        nc.scalar.copy(tst[:], acc[:])
                    row0 = HALF * h + P * r
                    nc.sync.dma_start(t_in[row0 : row0 + P, :], tst[:])
                    kt = 4 * h + r  # global strip index
                    if kt in (3, 5, 7):
                        t_subag({3: 0, 5: 1, 7: 2}[kt])

            # ---- phase 3: out_c = relu(max(a0*s0_c@t1 + b, a1*s2_c@t3 + b))
            def t_load(j, q, g):
                tq = strips.tile([P, D], cdt, name=f"tq_{q}", tag="strip")
                r0 = P * (q - GBASE[g])
                nc.sync.dma_start(tq[:], t_outs[g][r0 : r0 + P, :])
                if DEBUG:
                    nc.sync.dma_start(t_dump[P * j : P * (j + 1), :], tq[:])
                return tq

            HI_CHUNKS = [e for e in ARRIVAL if e[0] >= KCH]
            LO_CHUNKS = [e for e in ARRIVAL if e[0] < KCH]
            accs3 = [
                ps.tile([P, D], f32, name=f"acc3_{nt}", tag="acc")
                for nt in range(RCH)
            ]
            stash = [
                stashp.tile([P, D], f32, name=f"hst_{nt}", tag=f"hst{nt}")
                for nt in range(RCH)
            ]
            for nt in range(RCH):  # PSUM preload: bias/a1 for the high band
                if nt % 2 == 0:
                    nc.vector.tensor_copy(accs3[nt][:], bd1_128[:])
                else:
                    nc.scalar.copy(accs3[nt][:], bd1_128[:])
            for idx, (j, q, g, c) in enumerate(HI_CHUNKS):
                tq = t_load(j, q, g)
                jj = j - KCH
                strip = rstrips.tile([P, ROWS], cdt, name=f"rh_{q}", tag="strip")
                nc.sync.dma_start(strip[:], s2t[P * jj : P * (jj + 1), :])
                for nt in range(RCH):
                    nc.tensor.matmul(
                        accs3[nt][:],
                        mmcast(strip[:, P * nt : P * (nt + 1)]),
                        mmcast(tq[:]),
                        start=False,
                        stop=(idx == len(HI_CHUNKS) - 1),
                    )
            for nt in range(RCH):
                # stash = relu(a1*acc + b) fused; then preload bias/a0
                if nt % 2 == 0:
                    nc.vector.scalar_tensor_tensor(
                        stash[nt][:], accs3[nt][:], a128[:, 1:2], zeros[:],
                        mybir.AluOpType.mult, mybir.AluOpType.max,
                    )
                    nc.vector.tensor_copy(accs3[nt][:], bd0_128[:])
                else:
                    nc.scalar.mul(stash[nt][:], accs3[nt][:], a128[:, 1:2])
                    nc.scalar.activation(
                        stash[nt][:], stash[nt][:],
                        mybir.ActivationFunctionType.Relu,
                    )
                    nc.scalar.copy(accs3[nt][:], bd0_128[:])
            for idx, (j, q, g, c) in enumerate(LO_CHUNKS):
                tq = t_load(j, q, g)
                strip = rstrips.tile([P, ROWS], cdt, name=f"rl_{q}", tag="strip")
                nc.sync.dma_start(strip[:], s0t[P * j : P * (j + 1), :])
                for nt in range(RCH):
                    nc.tensor.matmul(
                        accs3[nt][:],
                        mmcast(strip[:, P * nt : P * (nt + 1)]),
                        mmcast(tq[:]),
                        start=False,
                        stop=(idx == len(LO_CHUNKS) - 1),
                    )
            for nt in range(RCH):
                # epilogue: relu(max(a0*lo+b, a1*hi+b)) == (acc*a0) max stash
                lo = stage.tile([P, D], f32, name=f"elo_{nt}", tag="elo")
                nc.vector.scalar_tensor_tensor(
                    lo[:], accs3[nt][:], a128[:, 0:1], stash[nt][:],
                    mybir.AluOpType.mult, mybir.AluOpType.max,
                )
                row0 = P * nt
                if nt % 2 == 0:
                    nc.sync.dma_start(out[row0 : row0 + P, :], lo[:])
                else:
                    nc.scalar.dma_start(out[row0 : row0 + P, :], lo[:])

    nc.compile()
    return nc


def _get_nc(compute):
    if compute not in _CACHE:
        _CACHE[compute] = _build_nc(compute)
    return _CACHE[compute]


def _shard_inputs(x, weights, alpha, bias, s0, s1, s2, s3, compute):
    import ml_dtypes

    cnp = ml_dtypes.bfloat16 if compute == "bf16" else np.float32

    def prep(a):  # transpose + cast, C-contiguous
        return np.ascontiguousarray(a.T).astype(cnp, copy=False)

    alpha = np.ascontiguousarray(alpha, dtype=np.float32)
    bias = np.ascontiguousarray(bias, dtype=np.float32)
    w_p = np.ascontiguousarray(weights).astype(cnp, copy=False)
    xn_full = np.ascontiguousarray(np.asarray(x)).astype(cnp, copy=False)
    in_maps = []
    for c in range(N_CORES):
        r0, r1 = ROWS * c, ROWS * (c + 1)
        if r1 <= K:
            s_rows = np.asarray(s1[r0:r1])
        elif r0 >= K:
            s_rows = np.asarray(s3[r0 - K : r1 - K])
        else:
            s_rows = np.concatenate([s1[r0:], s3[: r1 - K]], axis=0)
        in_maps.append(
            {
                "xn": xn_full,
                "w": w_p,
                "alpha": alpha,
                "bias": bias,
                "st": prep(s_rows),
                "s0t": prep(s0[r0:r1]),
                "s2t": prep(s2[r0:r1]),
            }
        )
    return in_maps


def kernel(x, weights, alpha, bias, s0, s1, s2, s3, _trace=False):
    from concourse.bass_utils import run_bass_kernel_spmd

    compute = COMPUTE
    nc = _get_nc(compute)
    in_maps = _shard_inputs(
        np.asarray(x), np.asarray(weights), np.asarray(alpha), np.asarray(bias),
        np.asarray(s0), np.asarray(s1), np.asarray(s2), np.asarray(s3), compute,
    )
    kwargs = {}
    if _trace:
        run_bass_kernel_spmd(nc, in_maps, core_ids=list(range(N_CORES)))
        kwargs = dict(trace=True, trace_cores=list(range(N_CORES)))
    r = run_bass_kernel_spmd(nc, in_maps, core_ids=list(range(N_CORES)), **kwargs)
    full = np.concatenate([res["out"] for res in r.results], axis=0)
    if _trace:
        return full, r
    return full


# revision 47
# speedup vs baseline: 1.0257x; 1.0257x over previous
"""SpGAT_Conv Trainium2 kernel: 8-core SPMD spectral GNN conv.

Math (reference):
    a = softmax(alpha)
    pre = x @ W                                   [N, D]
    out_low  = s0 @ (a0 * (s1 @ pre))             [N, D]
    out_high = s2 @ (a1 * (s3 @ pre))             [N, D]
    out = relu(max(out_low, out_high) + bias)

Key restructure (v15): associativity.  t = S @ (x @ W) == (S @ x) @ W.
The first form needs pre = x@W for ALL N rows on every core (replicated:
256 extra matmuls/core; sharding it needs a pre-AllGather that can never
be hidden).  The second form needs only U_c = S_c @ x -- and x is an
INPUT, already replicated for free -- then t_c = U_c @ W over the core's
own 1024 t rows (32 matmuls).  Same total FLOPs, zero new communication:
per-core matmul count drops 1286 -> ~1062.

Sharding: row-shard the node dim N across 8 cores.  S = concat(s1, s3).
Core c owns rows [1024c, 1024c+1024):
    phase 2a: U^T accumulation, contraction over x rows, in two 512-row
              halves (U^T half = 4 dchunks x [128, 512] = 4 PSUM banks);
              lhsT = x chunks (natural row-major!), rhs = S_c^T strips.
    phase 2b: t_half = (U^T)^T @ W via 16 matmuls from SBUF-drained U^T;
              strips staged and AllGathered (groups [0-3], [4,5], [6,7])
              as soon as their rows stage.
    phase 3:  out_c = relu(max(a0*s0_c@t1 + b, a1*s2_c@t3 + b)), high band
              then low band, bias/a preloaded into PSUM (continue-mode
              accumulation), high stash relu'd eagerly so the epilogue is
              a single fused (a0*acc) max stash DVE op per strip.

Queue discipline (hard-won): DMA HW queue ring slots form one global
sequence shared by both HWDGE engines, so any DMA blocked on a collective
semaphore stalls every later-slotted DMA.  Hence (a) t chunks are loaded
JIT in exact phase-3 consumption order -- whatever such a load stalls
needed that same collective anyway -- and (b) those loads share a tile
pool with the phase-2 strip loads so buffer-reuse deps keep them from
occupying ring slots until this core's phase 2 is nearly done.

Compute dtype is bf16 (host-cast; full PE rate) with fp32 PSUM
accumulation.  A dependency-free tiny AllGather at kernel start absorbs
first-collective init + inter-core launch skew.
"""

import os

import numpy as np

N_CORES = 8
N = 8192
K = 2048
NK = N - K          # 6144
D = 512
ROWS = N // N_CORES  # 1024 rows per core
P = 128
RCH = ROWS // P      # 8  (row chunks per core / output strips)
NCH = N // P         # 64 (contraction chunks over full N)
KCH = K // P         # 16 (low-band chunks; high band = 48)
DCH = D // P         # 4  (dchunks)
HALF = 512           # U rows per phase-2 half (4 PSUM banks of U^T)

# sub-AllGather strip groups (each collective costs ~25-35us mostly
# size-independent; the last gather is a hard barrier so its group is kept
# small and triggered as early as possible)
GROUPS = [[0, 1, 2, 3], [4, 5], [6, 7]]
GBASE = [0, 32, 48]  # q-index base of each group (8 ranks * strips)

COMPUTE = os.environ.get("SPGAT_COMPUTE", "bf16")
DEBUG = os.environ.get("SPGAT_DEBUG", "0") == "1"

_CACHE = {}

# t-chunk catalog: sub-AG g delivers, for every rank c, its strips GROUPS[g]
# = global chunks j = 8c + s.  Phase 3 consumes chunks in arrival (q) order:
# q = GBASE[g] + c * len(GROUPS[g]) + s_idx.
ARRIVAL = [
    (8 * c + s, GBASE[g] + c * len(GROUPS[g]) + si, g, c)
    for g in range(len(GROUPS))
    for c in range(N_CORES)
    for si, s in enumerate(GROUPS[g])
]


def _build_nc(compute):
    import concourse.mybir as mybir
    import concourse.tile as tile
    from concourse import bacc

    f32 = mybir.dt.float32
    bf16 = mybir.dt.bfloat16
    f32r = mybir.dt.float32r
    cdt = bf16 if compute == "bf16" else f32

    def mmcast(ap):
        return ap.bitcast(f32r) if compute == "f32r" else ap

    nc = bacc.Bacc(
        "TRN2", target_bir_lowering=False, debug=False, num_devices=N_CORES
    )

    xn = nc.dram_tensor("xn", [N, D], cdt, kind="ExternalInput").ap()
    w = nc.dram_tensor("w", [D, D], cdt, kind="ExternalInput").ap()
    alpha = nc.dram_tensor("alpha", [2], f32, kind="ExternalInput").ap()
    bias = nc.dram_tensor("bias", [D], f32, kind="ExternalInput").ap()
    st = nc.dram_tensor("st", [N, ROWS], cdt, kind="ExternalInput").ap()
    s0t = nc.dram_tensor("s0t", [K, ROWS], cdt, kind="ExternalInput").ap()
    s2t = nc.dram_tensor("s2t", [NK, ROWS], cdt, kind="ExternalInput").ap()
    out = nc.dram_tensor("out", [ROWS, D], f32, kind="ExternalOutput").ap()
    if DEBUG:
        t_dump = nc.dram_tensor("t_dump", [N, D], cdt, kind="ExternalOutput").ap()

    groups = [list(range(N_CORES))]

    with tile.TileContext(nc) as tc:
        with (
            tc.tile_pool(name="const", bufs=1) as const,
            tc.tile_pool(name="bigA", bufs=1) as bigA,
            tc.tile_pool(name="strips", bufs=12) as strips,
            tc.tile_pool(name="rstrips", bufs=12) as rstrips,
            tc.tile_pool(name="ut", bufs=8) as utp,
            tc.tile_pool(name="stage", bufs=5) as stage,
            tc.tile_pool(name="stash", bufs=1) as stashp,
            tc.tile_pool(name="ps", bufs=8, space="PSUM") as ps,
            tc.tile_pool(name="dram", bufs=1, space="DRAM") as dram,
        ):
            # ---- collective warm-up: absorb first-collective init + skew
            warm_in = dram.tile([8, 8], f32, name="warm_in")
            warm_out = dram.tile([64, 8], f32, name="warm_out", addr_space="Shared")
            nc.gpsimd.collective_compute(
                "AllGather",
                mybir.AluOpType.bypass,
                replica_groups=groups,
                ins=[warm_in.opt()],
                outs=[warm_out.opt()],
            )

            # ---- input DMAs: tiny alpha/bias first (feed the setup
            # matmuls, first on the PE queue), then w, then x chunked
            asb = const.tile([1, 2], f32, name="asb")
            nc.sync.dma_start(asb[:], alpha[None, :])
            bsb = const.tile([1, D], f32, name="bsb")
            nc.sync.dma_start(bsb[:], bias[None, :])
            w_sb = const.tile([P, DCH, D], cdt, name="w_sb")
            nc.sync.dma_start(w_sb[:], w.rearrange("(c p) d -> p c d", p=P))
            xn_v = xn.rearrange("(c p) d -> p c d", p=P)
            xn_sb = bigA.tile([P, NCH, D], cdt, name="xn_sb", tag="bigA")
            # only the first x piece is loaded up front; the rest interleave
            # with the phase-2a strip loads (issuing all 8MB here would queue
            # ahead of the first strip load and stall the PE ~30us)
            nc.sync.dma_start(xn_sb[:, 0:4, :], xn_v[:, 0:4, :])

            # ---- setup: softmax(alpha); broadcast a, bias/a0, bias/a1 to
            # 128 partitions via ones-matmul.
            amax = const.tile([1, 1], f32, name="amax")
            nc.vector.tensor_tensor(
                amax[:], asb[:, 0:1], asb[:, 1:2], mybir.AluOpType.max
            )
            ash = const.tile([1, 2], f32, name="ash")
            nc.vector.tensor_scalar(
                ash[:], asb[:], amax[:, 0:1], None, mybir.AluOpType.subtract
            )
            aexp = const.tile([1, 2], f32, name="aexp")
            nc.scalar.activation(aexp[:], ash[:], mybir.ActivationFunctionType.Exp)
            asum = const.tile([1, 1], f32, name="asum")
            nc.vector.tensor_tensor(
                asum[:], aexp[:, 0:1], aexp[:, 1:2], mybir.AluOpType.add
            )
            arec = const.tile([1, 1], f32, name="arec")
            nc.vector.reciprocal(arec[:], asum[:])
            afin = const.tile([1, 2], f32, name="afin")
            nc.vector.tensor_scalar(
                afin[:], aexp[:], arec[:, 0:1], None, mybir.AluOpType.mult
            )
            ainv = const.tile([1, 2], f32, name="ainv")
            nc.vector.reciprocal(ainv[:], afin[:])
            bd0 = const.tile([1, D], f32, name="bd0")
            nc.vector.tensor_scalar(
                bd0[:], bsb[:], ainv[:, 0:1], None, mybir.AluOpType.mult
            )
            bd1 = const.tile([1, D], f32, name="bd1")
            nc.vector.tensor_scalar(
                bd1[:], bsb[:], ainv[:, 1:2], None, mybir.AluOpType.mult
            )

            ones = const.tile([1, P], f32, name="ones")
            nc.vector.memset(ones[:], 1.0)
            zeros = const.tile([P, D], f32, name="zeros")
            nc.vector.memset(zeros[:], 0.0)
            ps_a = ps.tile([P, 2], f32, name="ps_a", tag="acc")
            nc.tensor.matmul(ps_a[:], ones[:], afin[:], start=True, stop=True)
            a128 = const.tile([P, 2], f32, name="a128")
            nc.vector.tensor_copy(a128[:], ps_a[:])
            ps_b = ps.tile([P, D], f32, name="ps_b", tag="acc")
            nc.tensor.matmul(ps_b[:], ones[:], bd0[:], start=True, stop=True)
            bd0_128 = const.tile([P, D], f32, name="bd0_128")
            nc.vector.tensor_copy(bd0_128[:], ps_b[:])
            ps_c = ps.tile([P, D], f32, name="ps_c", tag="acc")
            nc.tensor.matmul(ps_c[:], ones[:], bd1[:], start=True, stop=True)
            bd1_128 = const.tile([P, D], f32, name="bd1_128")
            nc.vector.tensor_copy(bd1_128[:], ps_c[:])

            # ---- phase 2: t_c = (S_c @ x) @ W in two 512-row halves
            t_in = dram.tile([ROWS, D], cdt, name="t_in")
            t_outs = [
                dram.tile([P * len(gs) * N_CORES, D], cdt, name=f"t_out{g}",
                          addr_space="Shared")
                for g, gs in enumerate(GROUPS)
            ]

            def t_subag(g):
                gs = GROUPS[g]
                nc.gpsimd.collective_compute(
                    "AllGather",
                    mybir.AluOpType.bypass,
                    replica_groups=groups,
                    ins=[t_in[P * gs[0] : P * (gs[-1] + 1), :].opt()],
                    outs=[t_outs[g].opt()],
                )

            for h in range(2):
                # --- 2a: U^T[dchunk][:, 512 rows] accumulation over x rows
                accU = [
                    ps.tile([P, HALF], f32, name=f"accU_{h}_{d}", tag="acc")
                    for d in range(DCH)
                ]
                for xc in range(NCH):
                    sl = strips.tile([P, HALF], cdt, name=f"st_{h}_{xc}",
                                     tag="strip")
                    nc.sync.dma_start(
                        sl[:],
                        st[P * xc : P * (xc + 1), HALF * h : HALF * (h + 1)],
                    )
                    if h == 0 and xc % 4 == 0 and xc // 4 + 1 < 16:
                        i = xc // 4 + 1  # stream the rest of x just ahead
                        nc.sync.dma_start(
                            xn_sb[:, 4 * i : 4 * (i + 1), :],
                            xn_v[:, 4 * i : 4 * (i + 1), :],
                        )
                    for d in range(DCH):
                        nc.tensor.matmul(
                            accU[d][:],
                            mmcast(xn_sb[:, xc, P * d : P * (d + 1)]),
                            mmcast(sl[:]),
                            start=(xc == 0),
                            stop=(xc == NCH - 1),
                        )
                uT = [
                    utp.tile([P, HALF], cdt, name=f"uT_{h}_{d}", tag="ut")
                    for d in range(DCH)
                ]
                for d in range(DCH):  # drain U^T to SBUF (bf16), alternate
                    if d % 2 == 0:
                        nc.vector.tensor_copy(uT[d][:], accU[d][:])
                    else:
                        nc.scalar.copy(uT[d][:], accU[d][:])
                # --- 2b: t rows [512h + 128r, +128) = U_half @ W
                for r in range(DCH):
                    acc = ps.tile([P, D], f32, name=f"acct_{h}_{r}", tag="acc")
                    for d in range(DCH):
                        nc.tensor.matmul(
                            acc[:],
                            mmcast(uT[d][:, P * r : P * (r + 1)]),
                            mmcast(w_sb[:, d, :]),
                            start=(d == 0),
                            stop=(d == DCH - 1),
                        )
                    tst = stage.tile([P, D], cdt, name=f"t_st_{h}_{r}", tag="st")
                    if r % 2 == 0:
                        nc.vector.tensor_copy(tst[:], acc[:])
                    else:
                        nc.scalar.copy(tst[:], acc[:])
                    row0 = HALF * h + P * r
                    nc.sync.dma_start(t_in[row0 : row0 + P, :], tst[:])
                    kt = 4 * h + r  # global strip index
                    if kt in (3, 5, 7):
                        t_subag({3: 0, 5: 1, 7: 2}[kt])

            # ---- phase 3: out_c = relu(max(a0*s0_c@t1 + b, a1*s2_c@t3 + b))
            def t_load(j, q, g):
                tq = strips.tile([P, D], cdt, name=f"tq_{q}", tag="strip")
                r0 = P * (q - GBASE[g])
                nc.sync.dma_start(tq[:], t_outs[g][r0 : r0 + P, :])
                if DEBUG:
                    nc.sync.dma_start(t_dump[P * j : P * (j + 1), :], tq[:])
                return tq

            HI_CHUNKS = [e for e in ARRIVAL if e[0] >= KCH]
            LO_CHUNKS = [e for e in ARRIVAL if e[0] < KCH]
            accs3 = [
                ps.tile([P, D], f32, name=f"acc3_{nt}", tag="acc")
                for nt in range(RCH)
            ]
            stash = [
                stashp.tile([P, D], f32, name=f"hst_{nt}", tag=f"hst{nt}")
                for nt in range(RCH)
            ]
            for nt in range(RCH):  # PSUM preload: bias/a1 for the high band
                if nt % 2 == 0:
                    nc.vector.tensor_copy(accs3[nt][:], bd1_128[:])
                else:
                    nc.scalar.copy(accs3[nt][:], bd1_128[:])
            for idx, (j, q, g, c) in enumerate(HI_CHUNKS):
                tq = t_load(j, q, g)
                jj = j - KCH
                strip = rstrips.tile([P, ROWS], cdt, name=f"rh_{q}", tag="strip")
                nc.sync.dma_start(strip[:], s2t[P * jj : P * (jj + 1), :])
                for nt in range(RCH):
                    nc.tensor.matmul(
                        accs3[nt][:],
                        mmcast(strip[:, P * nt : P * (nt + 1)]),
                        mmcast(tq[:]),
                        start=False,
                        stop=(idx == len(HI_CHUNKS) - 1),
                    )
            for nt in range(RCH):
                # stash = relu(a1*acc + b) fused; then preload bias/a0
                if nt % 2 == 0:
                    nc.vector.scalar_tensor_tensor(
                        stash[nt][:], accs3[nt][:], a128[:, 1:2], zeros[:],
                        mybir.AluOpType.mult, mybir.AluOpType.max,
                    )
                    nc.vector.tensor_copy(accs3[nt][:], bd0_128[:])
                else:
                    nc.scalar.mul(stash[nt][:], accs3[nt][:], a128[:, 1:2])
                    nc.scalar.activation(
                        stash[nt][:], stash[nt][:],
                        mybir.ActivationFunctionType.Relu,
                    )
                    nc.scalar.copy(accs3[nt][:], bd0_128[:])
            for idx, (j, q, g, c) in enumerate(LO_CHUNKS):
                tq = t_load(j, q, g)
                strip = rstrips.tile([P, ROWS], cdt, name=f"rl_{q}", tag="strip")
                nc.sync.dma_start(strip[:], s0t[P * j : P * (j + 1), :])
                for nt in range(RCH):
                    nc.tensor.matmul(
                        accs3[nt][:],
                        mmcast(strip[:, P * nt : P * (nt + 1)]),
                        mmcast(tq[:]),
                        start=False,
                        stop=(idx == len(LO_CHUNKS) - 1),
                    )
            for nt in range(RCH):
                # epilogue: relu(max(a0*lo+b, a1*hi+b)) == (acc*a0) max stash
                lo = stage.tile([P, D], f32, name=f"elo_{nt}", tag="elo")
                nc.vector.scalar_tensor_tensor(
                    lo[:], accs3[nt][:], a128[:, 0:1], stash[nt][:],
                    mybir.AluOpType.mult, mybir.AluOpType.max,
                )
                row0 = P * nt
                if nt % 2 == 0:
                    nc.sync.dma_start(out[row0 : row0 + P, :], lo[:])
                else:
                    nc.scalar.dma_start(out[row0 : row0 + P, :], lo[:])

    nc.compile()
    return nc


def _get_nc(compute):
    if compute not in _CACHE:
        _CACHE[compute] = _build_nc(compute)
    return _CACHE[compute]


def _shard_inputs(x, weights, alpha, bias, s0, s1, s2, s3, compute):
    import ml_dtypes

    cnp = ml_dtypes.bfloat16 if compute == "bf16" else np.float32

    def prep(a):  # transpose + cast, C-contiguous
        return np.ascontiguousarray(a.T).astype(cnp, copy=False)

    alpha = np.ascontiguousarray(alpha, dtype=np.float32)
    bias = np.ascontiguousarray(bias, dtype=np.float32)
    w_p = np.ascontiguousarray(weights).astype(cnp, copy=False)
    xn_full = np.ascontiguousarray(np.asarray(x)).astype(cnp, copy=False)
    in_maps = []
    for c in range(N_CORES):
        r0, r1 = ROWS * c, ROWS * (c + 1)
        if r1 <= K:
            s_rows = np.asarray(s1[r0:r1])
        elif r0 >= K:
            s_rows = np.asarray(s3[r0 - K : r1 - K])
        else:
            s_rows = np.concatenate([s1[r0:], s3[: r1 - K]], axis=0)
        in_maps.append(
            {
                "xn": xn_full,
                "w": w_p,
                "alpha": alpha,
                "bias": bias,
                "st": prep(s_rows),
                "s0t": prep(s0[r0:r1]),
                "s2t": prep(s2[r0:r1]),
            }
        )
    return in_maps


def kernel(x, weights, alpha, bias, s0, s1, s2, s3, _trace=False):
    from concourse.bass_utils import run_bass_kernel_spmd

    compute = COMPUTE
    nc = _get_nc(compute)
    in_maps = _shard_inputs(
        np.asarray(x), np.asarray(weights), np.asarray(alpha), np.asarray(bias),
        np.asarray(s0), np.asarray(s1), np.asarray(s2), np.asarray(s3), compute,
    )
    kwargs = {}
    if _trace:
        run_bass_kernel_spmd(nc, in_maps, core_ids=list(range(N_CORES)))
        kwargs = dict(trace=True, trace_cores=list(range(N_CORES)))
    r = run_bass_kernel_spmd(nc, in_maps, core_ids=list(range(N_CORES)), **kwargs)
    full = np.concatenate([res["out"] for res in r.results], axis=0)
    if _trace:
        return full, r
    return full
